# revision 10
# baseline (speedup 1.0000x reference)
"""Trainium2 Bass MHA kernel, head-sharded, v3.

Same math/work split as v2 (head A = head c over all 4096 queries; head B =
head 8+c//2 over a host-selected half; 12 cells x 512q x 4096k exps/core on
the scalar engine), with the schedule restructured around the ACT roofline:

 - x^T arrives via 8 column-block DMAs (all 6 row-chunks per block in one
   descriptor), so K/Q projection chunk b only waits for column block b and
   the exp stream starts ~7us in.
 - B cells run FIRST; their AllToAll (0.5MB) fires mid-kernel and is fully
   hidden under the A cells, as is the B-half of the FC (contraction chunks
   4,5 -> osbB).  Only the A-half AllToAll + 4-chunk FC + add remain in the
   tail.
 - Cells are emitted under tc.high_priority: projections (KTb/QTb first,
   then V, then KTa/QTa) and the FC-B run as PE fill work in the cells' ACT
   slack.  A 16-deep exp-tile pool decouples the exp stream from the AV
   matmuls; a dedicated PSUM bank for the reciprocal broadcast keeps the
   normalize off the fill-work PSUM rotation.
"""

import numpy as np
import ml_dtypes

import concourse.bass as bass
import concourse.tile as tile
from concourse import mybir
import bass_rust

HID = 768
S = 4096
NCORES = 8
SC = 512                  # queries per attention cell
HEADS = 12
HD = 64
NCH = HID // 128          # 6 feature chunks
NKT = S // 128            # 32 kpos blocks
NQB = SC // 128
SCALE = 1.0 / 8.0

BF = mybir.dt.bfloat16
F32 = mybir.dt.float32
U8 = mybir.dt.uint8
EXP = mybir.ActivationFunctionType.Exp
MUL = mybir.AluOpType.mult
ADD = mybir.AluOpType.add

VB = 130                  # per kpos block: [A 64 | ones | B 64 | ones]
SH = 64 * SC              # per-sender shard elems in each A2A buffer
CELL_PRIO = 10 ** 6


def split_excess_waits(nc, max_waits=1):
    """This walrus build rejects >1 sem wait per instruction; move extras
    onto preceding NOPs on the same engine (same semantics: engine blocks
    until all waits pass before executing the original instruction)."""
    ctr = 0
    for fn in nc.m.functions:
        for bb in fn.blocks:
            new_list = []
            for ins in bb.instructions:
                si = ins.sync_info
                if si is not None and si.on_wait and len(si.on_wait) > max_waits:
                    waits = list(si.on_wait)
                    while len(waits) > max_waits:
                        chunk, waits = waits[:max_waits], waits[max_waits:]
                        nop = bass_rust.InstNoOp(
                            name=f"I-waitsplit-{ctr}", ins=[], outs=[])
                        ctr += 1
                        nop.engine = ins.engine
                        nop.sync_info = mybir.SyncInfo(on_wait=chunk, on_update=[])
                        new_list.append(nop)
                    ins.sync_info = mybir.SyncInfo(
                        on_wait=waits, on_update=list(si.on_update))
                new_list.append(ins)
            bb.instructions[:] = new_list
    return ctr


def build_nc(split_waits=True, repeats=1, no_collective=False):
    nc = bass.Bass()
    xT = nc.declare_dram_parameter("xT", [HID, S], BF, isOutput=False)
    xqBT = nc.declare_dram_parameter("xqBT", [HID, S // 2], BF, isOutput=False)
    wqA = nc.declare_dram_parameter("wqA", [HID, 128], BF, isOutput=False)
    wkA = nc.declare_dram_parameter("wkA", [HID, 128], BF, isOutput=False)
    wqB = nc.declare_dram_parameter("wqB", [HID, 128], BF, isOutput=False)
    wkB = nc.declare_dram_parameter("wkB", [HID, 128], BF, isOutput=False)
    wvAB = nc.declare_dram_parameter("wvAB", [HID, 128], BF, isOutput=False)
    wfcT = nc.declare_dram_parameter("wfcT", [HID, HID], BF, isOutput=False)
    mE = nc.declare_dram_parameter("mE", [128, SC], U8, isOutput=False)
    out = nc.declare_dram_parameter("out", [SC, HID], F32, isOutput=True)

    with tile.TileContext(nc) as tc:
        with (
            tc.tile_pool(name="x", bufs=1) as p_x,
            tc.tile_pool(name="w", bufs=1) as p_w,
            tc.tile_pool(name="proj", bufs=1) as p_proj,
            tc.tile_pool(name="oNT", bufs=NCH) as p_oNT,
            tc.tile_pool(name="expT", bufs=14) as p_exp,
            tc.tile_pool(name="norm", bufs=2) as p_norm,
            tc.tile_pool(name="misc", bufs=1) as p_misc,
            tc.tile_pool(name="osb", bufs=2) as p_osb,
            tc.tile_pool(name="dram", bufs=1, space="DRAM") as p_dram,
            tc.tile_pool(name="pp_mm", bufs=2, space="PSUM") as pp_mm,
            tc.tile_pool(name="pp_sc", bufs=2, space="PSUM") as pp_sc,
            tc.tile_pool(name="pp_acc", bufs=1, space="PSUM") as pp_acc,
            tc.tile_pool(name="pp_rbp", bufs=1, space="PSUM") as pp_rbp,
        ):
            for _rep in range(repeats):
                # ---- loads ----
                # x in 8 column blocks (each lands all 6 row-chunks), so
                # projection chunk b only waits on column block b.
                def loadw6(param, nm, cols=128):
                    w6 = p_w.tile([128, NCH * cols], BF, tag=f"w{nm}",
                                  bufs=1, name=f"w{nm}")
                    nc.sync.dma_start(
                        out=w6.rearrange("p (j c) -> p j c", j=NCH),
                        in_=param.rearrange("(j p) c -> p j c", p=128))
                    return w6

                # B-head weights first (they gate the first exps), then x
                # column block 0, then the rest of x, then the cold weights.
                wkb6 = loadw6(wkB, "kb")
                wqb6 = loadw6(wqB, "qb")
                x6 = p_x.tile([128, NCH * S], BF, tag="x6", bufs=1,
                              name="x6")
                xv = x6.rearrange("p (j c) -> p j c", j=NCH)
                xTv = xT.rearrange("(j p) c -> p j c", p=128)
                xb6 = p_x.tile([128, NCH * (S // 2)], BF, tag="xb6", bufs=1,
                               name="xb6")
                xbv = xb6.rearrange("p (j c) -> p j c", j=NCH)
                xqv = xqBT.rearrange("(j p) c -> p j c", p=128)

                def load_x_block(b):
                    nc.sync.dma_start(
                        out=xv[:, :, 512 * b:512 * (b + 1)],
                        in_=xTv[:, :, 512 * b:512 * (b + 1)])

                def load_xb_block(b):
                    nc.sync.dma_start(
                        out=xbv[:, :, 1024 * b:1024 * (b + 1)],
                        in_=xqv[:, :, 1024 * b:1024 * (b + 1)])

                load_x_block(0)
                load_xb_block(0)
                load_x_block(1)
                wv6 = loadw6(wvAB, "v")
                load_x_block(2)
                load_xb_block(1)
                for b in range(3, 8):
                    load_x_block(b)
                wka6 = loadw6(wkA, "ka")
                wqa6 = loadw6(wqA, "qa")
                wfc6 = loadw6(wfcT, "fc", cols=HID)

                me_sb = p_misc.tile([128, SC], U8, tag="mE", name="mEs")
                nc.sync.dma_start(out=me_sb, in_=mE[:, :])
                ones_sb = p_misc.tile([1, HD], BF, tag="ones", name="ones")
                nc.vector.memset(ones_sb, 1.0)

                # ---- projections ----
                def proj_chunk(w6, src6, n_src, dst, b, nm, cols=128):
                    ps = pp_mm.tile([128, 512], F32, tag="mm",
                                    name=f"ps{nm}{b}")
                    for i in range(NCH):
                        nc.tensor.matmul(
                            ps,
                            lhsT=w6[:, cols * i:cols * (i + 1)],
                            rhs=src6[:, n_src * i + 512 * b:
                                     n_src * i + 512 * (b + 1)],
                            start=(i == 0), stop=(i == NCH - 1))
                    nc.vector.tensor_copy(dst[:, 512 * b:512 * (b + 1)], ps)

                KTb = p_proj.tile([128, S], BF, tag="KTb", bufs=1, name="KTb")
                QTb = p_proj.tile([128, S // 2], BF, tag="QTb", bufs=1,
                                  name="QTb")
                for b in range(S // 512):
                    proj_chunk(wkb6, x6, S, KTb, b, "KTb")
                    if b < 4:
                        proj_chunk(wqb6, xb6, S // 2, QTb, b, "QTb")

                # V for both heads: [128 kpos, 32*130], [A 64|1|B 64|1]
                VT = p_proj.tile([128, NKT * VB], BF, tag="VT", bufs=1,
                                 name="VT")
                v4 = VT.rearrange("p (t a b) -> p t a b", a=2, b=65)
                nc.vector.memset(v4[:, :, :, HD:HD + 1], 1.0)
                for t in range(NKT):
                    ps = pp_mm.tile([128, 128], F32, tag="mm",
                                    name=f"psv{t}")
                    for i in range(NCH):
                        nc.tensor.matmul(
                            ps,
                            lhsT=x6[:, S * i + 128 * t:S * i + 128 * (t + 1)],
                            rhs=wv6[:, 128 * i:128 * (i + 1)],
                            start=(i == 0), stop=(i == NCH - 1))
                    dst = VT[:, VB * t:VB * t + VB] \
                        .rearrange("p (a b) -> p a b", a=2)[:, :, 0:HD]
                    src = ps[:, 0:128].rearrange("p (a b) -> p a b", a=2)
                    nc.vector.tensor_copy(dst, src)

                KTa = p_proj.tile([128, S], BF, tag="KTa", bufs=1, name="KTa")
                QTa = p_proj.tile([128, S], BF, tag="QTa", bufs=1, name="QTa")
                for b in range(S // 512):
                    proj_chunk(wka6, x6, S, KTa, b, "KTa")
                    proj_chunk(wqa6, x6, S, QTa, b, "QTa")

                # ---- A2A buffers ----
                aB_in = p_dram.tile([NCORES * SH], BF, name=f"aBi{_rep}")
                aB_out = p_dram.tile([NCORES * SH], BF, name=f"aBo{_rep}")
                aA_in = p_dram.tile([NCORES * SH], BF, name=f"aAi{_rep}")
                aA_out = p_dram.tile([NCORES * SH], BF, name=f"aAo{_rep}")

                # ---- attention cells ----
                def cell(KT, QT, vh, qb, shards, buf):
                    acc = pp_acc.tile([128, SC], F32, tag="acc",
                                      name=f"acc{vh}_{qb}")
                    for ti in range(NKT // 2):
                        t = 2 * ti
                        sc = pp_sc.tile([128, 2 * SC], F32, tag="sc",
                                        name=f"sc{vh}_{qb}_{ti}")
                        for half in range(2):
                            nc.tensor.matmul(
                                sc[:, SC * half:SC * (half + 1)],
                                lhsT=KT[64 * half:64 * (half + 1),
                                        128 * (t + half):128 * (t + half + 1)],
                                rhs=QT[64 * half:64 * (half + 1),
                                       SC * qb:SC * (qb + 1)],
                                start=True, stop=True)
                        et = p_exp.tile([128, 2 * SC], BF, tag="expT",
                                        name=f"et{vh}_{qb}_{ti}")
                        nc.scalar.activation(et, sc, EXP, scale=SCALE)
                        for half in range(2):
                            nc.tensor.matmul(
                                acc[0:HD + 1, :],
                                lhsT=VT[:, VB * (t + half) + 65 * vh:
                                        VB * (t + half) + 65 * vh + HD + 1],
                                rhs=et[:, SC * half:SC * (half + 1)],
                                start=(ti == 0 and half == 0),
                                stop=(ti == NKT // 2 - 1 and half == 1))
                    # stage acc out of PSUM in one copy (frees the
                    # single acc bank for the next cell), then normalize:
                    # on = a_dims * broadcast(1/denom)
                    a = p_norm.tile([HD + 1, SC], F32, tag="accs", bufs=2,
                                    name=f"as{vh}_{qb}")
                    nc.vector.tensor_copy(a, acc[0:HD + 1, :])
                    rc = p_norm.tile([1, SC], BF, tag="recip", bufs=2,
                                     name=f"rc{vh}_{qb}")
                    with nc.allow_low_precision(
                            reason="softmax denom reciprocal in bf16"):
                        nc.vector.reciprocal(rc, a[HD:HD + 1, :])
                    rbp = pp_rbp.tile([128, SC], F32, tag="rbp",
                                      name=f"rb{vh}_{qb}")
                    nc.tensor.matmul(rbp[0:HD, :], lhsT=ones_sb, rhs=rc,
                                     start=True, stop=True)
                    rb = p_norm.tile([64, SC], BF, tag="rb", bufs=2,
                                     name=f"rbs{vh}_{qb}")
                    nc.vector.tensor_copy(rb, rbp[0:HD, :])
                    on = p_norm.tile([64, SC], BF, tag="on", bufs=2,
                                     name=f"on{vh}_{qb}")
                    nc.vector.scalar_tensor_tensor(
                        out=on, in0=a[0:HD, :], scalar=1.0,
                        in1=rb, op0=MUL, op1=MUL)
                    for shard in shards:
                        nc.sync.dma_start(
                            out=buf[shard * SH:(shard + 1) * SH]
                            .rearrange("(p n) -> p n", p=64),
                            in_=on)

                for qb in range(4):          # head B cells first
                    with tc.high_priority(CELL_PRIO):
                        cell(KTb, QTb, 1, qb, [qb, qb + 4], aB_in)

                if no_collective:
                    aB_out = aB_in      # timing diagnostic: wrong numerics
                else:
                    nc.gpsimd.collective_compute(
                        "AllToAll",
                        mybir.AluOpType.bypass,
                        replica_groups=[list(range(NCORES))],
                        ins=[aB_in],
                        outs=[aB_out],
                    )

                # assemble heads 8-11 chunks (oNT[4], oNT[5]) + FC B-half
                oNT_B = []
                for g in range(2):
                    E = p_norm.tile([128, SC], BF, tag="Eb", bufs=2,
                                    name=f"E{g}")
                    O = p_norm.tile([128, SC], BF, tag="Ob", bufs=2,
                                    name=f"O{g}")
                    for half in range(2):
                        se = 4 * g + 2 * half
                        nc.sync.dma_start(
                            out=E[64 * half:64 * (half + 1), :],
                            in_=aB_out[se * SH:(se + 1) * SH]
                            .rearrange("(p n) -> p n", p=64))
                        nc.sync.dma_start(
                            out=O[64 * half:64 * (half + 1), :],
                            in_=aB_out[(se + 1) * SH:(se + 2) * SH]
                            .rearrange("(p n) -> p n", p=64))
                    ot = p_oNT.tile([128, SC], BF, tag="oNT",
                                    name=f"oNTb{g}")
                    nc.vector.tensor_copy(ot, O)
                    nc.vector.copy_predicated(ot, me_sb, E)
                    oNT_B.append(ot)

                osbB = []
                for qb in range(NQB):
                    ob = p_osb.tile([128, HID], F32, tag="osbB", bufs=NQB,
                                    name=f"osbB{qb}")
                    for fh in range(2):
                        ps = pp_mm.tile([128, 384], F32, tag="mm",
                                        name=f"psb{qb}_{fh}")
                        for g in range(2):
                            j = 4 + g
                            nc.tensor.matmul(
                                ps,
                                lhsT=oNT_B[g][:, 128 * qb:128 * (qb + 1)],
                                rhs=wfc6[:, HID * j + 384 * fh:
                                         HID * j + 384 * (fh + 1)],
                                start=(g == 0), stop=(g == 1))
                        nc.vector.tensor_copy(
                            ob[:, 384 * fh:384 * (fh + 1)], ps)
                    osbB.append(ob)

                for qb in range(8):          # head A cells
                    with tc.high_priority(CELL_PRIO):
                        cell(KTa, QTa, 0, qb, [qb], aA_in)

                if no_collective:
                    aA_out = aA_in
                else:
                    nc.gpsimd.collective_compute(
                        "AllToAll",
                        mybir.AluOpType.bypass,
                        replica_groups=[list(range(NCORES))],
                        ins=[aA_in],
                        outs=[aA_out],
                    )

                # assemble heads 0-7 chunks + FC A-half + combine + store
                oNT_A = []
                for f in range(4):
                    ot = p_oNT.tile([128, SC], BF, tag="oNT", name=f"oNT{f}")
                    for half in range(2):
                        h = 2 * f + half
                        nc.sync.dma_start(
                            out=ot[64 * half:64 * (half + 1), :],
                            in_=aA_out[h * SH:(h + 1) * SH]
                            .rearrange("(p n) -> p n", p=64))
                    oNT_A.append(ot)
                for qb in range(NQB):
                    osb = p_osb.tile([128, HID], F32, tag="osb", bufs=2,
                                     name=f"osb{qb}")
                    for fh in range(2):
                        ps = pp_mm.tile([128, 384], F32, tag="mm",
                                        name=f"psf{qb}_{fh}")
                        for j in range(4):
                            nc.tensor.matmul(
                                ps,
                                lhsT=oNT_A[j][:, 128 * qb:128 * (qb + 1)],
                                rhs=wfc6[:, HID * j + 384 * fh:
                                         HID * j + 384 * (fh + 1)],
                                start=(j == 0), stop=(j == 3))
                        nc.vector.scalar_tensor_tensor(
                            out=osb[:, 384 * fh:384 * (fh + 1)],
                            in0=ps, scalar=1.0,
                            in1=osbB[qb][:, 384 * fh:384 * (fh + 1)],
                            op0=MUL, op1=ADD)
                    nc.sync.dma_start(
                        out=out[128 * qb:128 * (qb + 1), :], in_=osb)

    if split_waits:
        split_excess_waits(nc)
    return nc


_NC_CACHE = None


def _get_nc():
    global _NC_CACHE
    if _NC_CACHE is None:
        _NC_CACHE = build_nc()
    return _NC_CACHE


def make_in_maps(x, w_q, w_k, w_v, w_fc):
    bf16 = ml_dtypes.bfloat16
    xTf = np.ascontiguousarray(np.asarray(x, np.float32)[0].T).astype(bf16)
    wqT = np.asarray(w_q, np.float32).T.astype(bf16)
    wkT = np.asarray(w_k, np.float32).T.astype(bf16)
    wvT = np.asarray(w_v, np.float32).T.astype(bf16)
    wfc = np.ascontiguousarray(np.asarray(w_fc, np.float32).T).astype(bf16)
    in_maps = []
    for c in range(NCORES):
        A, B, half = c, 8 + c // 2, c % 2
        sa, sb = slice(64 * A, 64 * A + 64), slice(64 * B, 64 * B + 64)

        def dup(w, s):
            return np.ascontiguousarray(
                np.concatenate([w[:, s], w[:, s]], axis=1))
        m_e = np.full((128, SC), 1 if c < 4 else 0, np.uint8)
        in_maps.append({
            "xT": np.ascontiguousarray(xTf),
            "xqBT": np.ascontiguousarray(
                xTf[:, 2048 * half:2048 * (half + 1)]),
            "wqA": dup(wqT, sa), "wkA": dup(wkT, sa),
            "wqB": dup(wqT, sb), "wkB": dup(wkT, sb),
            "wvAB": np.ascontiguousarray(
                np.concatenate([wvT[:, sa], wvT[:, sb]], axis=1)),
            "wfcT": wfc,
            "mE": m_e,
        })
    return in_maps


def kernel(x, w_q, w_k, w_v, w_fc):
    from concourse.bass_utils import run_bass_kernel_spmd
    nc = _get_nc()
    in_maps = make_in_maps(x, w_q, w_k, w_v, w_fc)
    res = run_bass_kernel_spmd(nc, in_maps, core_ids=list(range(NCORES)))
    out = np.concatenate([res.results[c]["out"] for c in range(NCORES)],
                         axis=0)
    return out.reshape(1, S, HID).astype(np.float32)
